# revision 9
# baseline (speedup 1.0000x reference)
"""Trainium2 Bass kernel for ConditionalGMM loss (nll mean + argmax assignment).

Math: with univariate Gaussians per dim,
  mix_p[b,k] = log_softmax(pi)[k] + sum_d [ a*f_lp + (1-a)*b_lp ]
Expanding the squared terms turns the big sum over d into two matmuls:
  mix_p'[b,k] = sum_d x^2[b,d]*(-0.5*W2[k,d]) + sum_d x[b,d]*W1[k,d] + c0[k]
with the background (b_*) terms folded in as an extra matmul output column
carrying the k-independent per-sample term Sb[b].
Sharding: data-parallel over batch, 128 rows per core on 8 cores; (K,D)
parameters replicated.  Assignment = argmax_k, nll = -(Sb + logsumexp_k).
"""

import sys

sys.path.insert(0, "/opt/trn_rl_repo")

import numpy as np

B, D, K = 1024, 2000, 32
NCORES = 8
P = B // NCORES  # 128 batch rows per core
CD, CH = 125, 16  # contraction chunks: D = CH * CD
PR = 98  # stacked param rows: f_D(32) f_mu(32) alpha(32) b_D(1) b_mu(1)
NW = K + 1  # matmul rhs cols: 32 mixture cols + 1 background (Sb) col
LOG2PI = float(np.log(2.0 * np.pi))

_CACHE = {}


def _build(loop_iters=None):
    import concourse.bass as bass
    import concourse.bacc as bacc
    import concourse.mybir as mybir
    from concourse.tile import TileContext

    f32 = mybir.dt.float32
    u32 = mybir.dt.uint32
    Act = mybir.ActivationFunctionType
    Alu = mybir.AluOpType
    AX = mybir.AxisListType

    nc = bacc.Bacc()

    xs_d = nc.dram_tensor("xs", [P, D], f32, kind="ExternalInput").ap()
    fst_d = nc.dram_tensor("fstack", [PR, D], f32, kind="ExternalInput").ap()
    pi_d = nc.dram_tensor("pi_row", [1, K], f32, kind="ExternalInput").ap()
    onec_d = nc.dram_tensor("ones_col", [P, 1], f32, kind="ExternalInput").ap()
    oner_d = nc.dram_tensor("ones_row", [1, P], f32, kind="ExternalInput").ap()
    id_d = nc.dram_tensor("ident", [P, P], f32, kind="ExternalInput").ap()
    loss_d = nc.dram_tensor("loss_acc", [1, 1], f32, kind="ExternalOutput").ap()
    idx_d = nc.dram_tensor("idx_out", [P, 1], u32, kind="ExternalOutput").ap()

    act = nc.scalar
    dve = nc.vector

    with TileContext(nc) as tc:
        with (
            tc.tile_pool(name="const", bufs=1) as cp,
            tc.tile_pool(name="par", bufs=1) as pp,
            tc.tile_pool(name="xp", bufs=1) as xp,
            tc.tile_pool(name="xs3", bufs=3) as xq,
            tc.tile_pool(name="ep", bufs=1) as ep,
            tc.tile_pool(name="pst", bufs=3, space="PSUM") as pst,
            tc.tile_pool(name="psm", bufs=1, space="PSUM") as psm,
            tc.tile_pool(name="psw", bufs=1, space="PSUM") as psw,
        ):
            # ---- one-time constants ---------------------------------------
            ident = cp.tile([P, P], f32, tag="ident", name="ident")
            nc.sync.dma_start(ident[:], id_d)
            onec = cp.tile([P, 1], f32, tag="onec", name="onec")
            nc.sync.dma_start(onec[:], onec_d)
            oner = cp.tile([1, P], f32, tag="oner", name="oner")
            nc.sync.dma_start(oner[:], oner_d)

            # warm the ACT natural_log_exp table ASAP (no DMA dependency)
            wsrc = cp.tile([1, 1], f32, tag="wsrc", name="wsrc")
            nc.gpsimd.memset(wsrc[:], 0)
            warm = cp.tile([1, 1], f32, tag="warm", name="warm")
            act.activation(warm[:], wsrc[:], Act.Exp)

            def body():
                # ---- parameter load + transpose ---------------------------
                pi_r = cp.tile([1, K], f32, tag="pi", name="pi")
                nc.sync.dma_start(pi_r[:], pi_d)
                praw = pp.tile([PR, D], f32, tag="praw", name="praw")
                for j in range(2):
                    nc.sync.dma_start(
                        praw[:, j * 1000 : (j + 1) * 1000],
                        fst_d[:, j * 1000 : (j + 1) * 1000],
                    )
                # parT[:, t*PR + j] = param row j at dim d = t*CD + p
                parT = pp.tile([CD, CH * PR], f32, tag="parT", name="parT")
                for t in range(CH):
                    pt = pst.tile([CD, P], f32, tag="tp", name="tp")
                    nc.tensor.transpose(
                        pt[:, 0:PR], praw[:, t * CD : (t + 1) * CD], ident[0:PR, 0:PR]
                    )
                    act.copy(parT[:, t * PR : (t + 1) * PR], pt[:, 0:PR])

                def pview(j0, w):
                    # [CD, CH, w] view of param rows j0..j0+w across chunks
                    a = parT[:]
                    return bass.AP(
                        a.tensor, a.offset + j0, [a.ap[0], [PR, CH], [1, w]]
                    )

                fDT = pview(0, K)
                fmuT = pview(K, K)
                alT = pview(2 * K, K)
                bDT = pview(3 * K, 1)
                bmuT = pview(3 * K + 1, 1)

                def big(tag):
                    return pp.tile([CD, CH * K], f32, tag=tag, name=tag)

                def v3(ap):
                    # [CD, CH*K] contiguous -> [CD, CH, K]
                    return bass.AP(ap.tensor, ap.offset, [ap.ap[0], [K, CH], [1, K]])

                # f_var = softplus(f_D) = ln(1+exp(f_D)); a = sigmoid(10*alpha)
                eA = big("eA")
                act.activation(v3(eA[:]), fDT, Act.Exp)
                s1 = big("s1")
                dve.tensor_scalar_add(s1[:], eA[:], 1.0)
                fv = big("fv")
                act.activation(fv[:], s1[:], Act.Ln)
                rf = big("rf")
                dve.reciprocal(rf[:], fv[:])
                lvf = big("lvf")
                act.activation(lvf[:], fv[:], Act.Ln)
                eB = big("eB")
                act.activation(v3(eB[:]), alT, Act.Exp, scale=-10.0)
                s2 = big("s2")
                dve.tensor_scalar_add(s2[:], eB[:], 1.0)
                aS = big("aS")
                dve.reciprocal(aS[:], s2[:])
                P2 = big("P2")
                dve.tensor_mul(P2[:], aS[:], rf[:])
                P1 = big("P1")
                dve.tensor_mul(P1[:], P2[:], fmuT)
                t0 = big("t0")
                dve.tensor_mul(t0[:], P1[:], fmuT)
                u0 = big("u0")
                dve.tensor_mul(u0[:], aS[:], lvf[:])
                A0 = big("A0")  # a*(f_mu^2*r_f + ln f_var)
                dve.tensor_add(A0[:], t0[:], u0[:])

                # ---- background (b_*) vectors, [CD, CH] -------------------
                def sml(tag):
                    return pp.tile([CD, CH], f32, tag=tag, name=tag)

                bve = sml("bve")
                act.activation(bve[:], bDT, Act.Exp)
                bs1 = sml("bs1")
                dve.tensor_scalar_add(bs1[:], bve[:], 1.0)
                bv = sml("bv")
                act.activation(bv[:], bs1[:], Act.Ln)
                rb = sml("rb")
                dve.reciprocal(rb[:], bv[:])
                lvb = sml("lvb")
                act.activation(lvb[:], bv[:], Act.Ln)
                hrb = sml("hrb")  # 0.5 * r_b
                act.mul(hrb[:], rb[:], 0.5)
                nbr = sml("nbr")  # -b_mu * r_b
                dve.scalar_tensor_tensor(
                    nbr[:], bmuT, -1.0, rb[:], Alu.mult, Alu.mult
                )
                q2 = sml("q2")  # b_mu^2 * r_b
                b2 = sml("b2")
                act.square(b2[:], bmuT)
                dve.tensor_mul(q2[:], b2[:], rb[:])
                nqb = sml("nqb")  # -(b_mu^2 r_b + ln b_var)
                nqbrow = pp.tile([CD, 1], f32, tag="nqbrow", name="nqbrow")
                dve.scalar_tensor_tensor(
                    nqb[:],
                    q2[:],
                    -1.0,
                    lvb[:],
                    Alu.mult,
                    Alu.subtract,
                    accum_out=nqbrow[:],
                )

                # ---- m1 = a*(0.5 r_b), m2 = a*(-b_mu r_b) (bcast over k) --
                def bc0(ap):
                    return bass.AP(
                        ap.tensor, ap.offset, [ap.ap[0], [1, CH], [0, K]]
                    )

                m1 = big("m1")
                dve.tensor_mul(m1[:], aS[:], bc0(hrb[:]))
                m2 = big("m2")
                dve.tensor_mul(m2[:], aS[:], bc0(nbr[:]))

                # ---- matmul rhs tensors [CD, CH*NW] -----------------------
                rhs2 = pp.tile([CD, CH * NW], f32, tag="rhs2", name="rhs2")
                rhs1 = pp.tile([CD, CH * NW], f32, tag="rhs1", name="rhs1")

                def wview(ap):
                    return bass.AP(
                        ap.tensor, ap.offset, [ap.ap[0], [NW, CH], [1, K]]
                    )

                def bcolv(ap):
                    return bass.AP(ap.tensor, ap.offset + K, [ap.ap[0], [NW, CH]])

                # rhs2 W-cols = -0.5*P2 + m1 ; b-col = -0.5*r_b
                dve.scalar_tensor_tensor(
                    wview(rhs2[:]), v3(P2[:]), -0.5, v3(m1[:]), Alu.mult, Alu.add
                )
                act.mul(bcolv(rhs2[:]), rb[:], -0.5)
                # rhs1 W-cols = P1 + m2 ; b-col = +b_mu*r_b = -nbr
                dve.scalar_tensor_tensor(
                    wview(rhs1[:]), v3(P1[:]), 1.0, v3(m2[:]), Alu.mult, Alu.add
                )
                act.mul(bcolv(rhs1[:]), nbr[:], -1.0)

                # ---- W0[k] = sum_d A0 - sum_d a*qb'  (PSUM [K,1]) ---------
                w0p = psw.tile([K, 1], f32, tag="w0p", name="w0p")
                for t in range(CH):
                    nc.tensor.matmul(
                        w0p[:],
                        A0[:, t * K : (t + 1) * K],
                        onec[0:CD, :],
                        start=(t == 0),
                        stop=False,
                    )
                    nc.tensor.matmul(
                        w0p[:],
                        aS[:, t * K : (t + 1) * K],
                        nqb[:, t : t + 1],
                        start=False,
                        stop=(t == CH - 1),
                    )
                w0s = ep.tile([K, 1], f32, tag="w0s", name="w0s")
                act.copy(w0s[:], w0p[:])
                w0r = psw.tile([1, K], f32, tag="w0r", name="w0r")
                nc.tensor.transpose(w0r[:], w0s[:], ident[0:K, 0:K])

                # sum_d nqb (PSUM [1,1]) for the background constant
                s0p = psw.tile([1, 1], f32, tag="tiny", name="tiny")
                nc.tensor.matmul(
                    s0p[:], nqbrow[:], onec[0:CD, :], start=True, stop=True
                )

                # ---- c0 row [1, NW]: log_softmax(pi) - 0.5*W0 | s0 --------
                mx = ep.tile([1, 1], f32, tag="mx", name="mx")
                dve.tensor_reduce(mx[:], pi_r[:], AX.X, Alu.max)
                g1 = ep.tile([1, K], f32, tag="g1", name="g1")
                dve.tensor_scalar_sub(g1[:], pi_r[:], mx[:])
                epi = ep.tile([1, K], f32, tag="epi", name="epi")
                spi = ep.tile([1, 1], f32, tag="spi", name="spi")
                act.activation(epi[:], g1[:], Act.Exp, accum_out=spi[:])
                lnspi = ep.tile([1, 1], f32, tag="lnspi", name="lnspi")
                act.activation(lnspi[:], spi[:], Act.Ln)
                g2 = ep.tile([1, K], f32, tag="g2", name="g2")
                dve.tensor_scalar_sub(g2[:], g1[:], lnspi[:])
                c0 = ep.tile([1, NW], f32, tag="c0", name="c0")
                dve.scalar_tensor_tensor(
                    c0[:, 0:K], w0r[:], -0.5, g2[:], Alu.mult, Alu.add
                )
                act.activation(
                    c0[:, K : K + 1],
                    s0p[:],
                    Act.Copy,
                    bias=-0.5 * D * LOG2PI,
                    scale=0.5,
                )

                # ---- x: load, transpose, square, matmul-accumulate --------
                xnat = xp.tile([P, D], f32, tag="xnat", name="xnat")
                for j in range(4):
                    nc.sync.dma_start(
                        xnat[:, j * 500 : (j + 1) * 500],
                        xs_d[:, j * 500 : (j + 1) * 500],
                    )
                mixp = psm.tile([P, NW], f32, tag="mix", name="mix")
                for t in range(CH):
                    xpt = pst.tile([CD, P], f32, tag="tp", name="tp")
                    nc.tensor.transpose(
                        xpt[:], xnat[:, t * CD : (t + 1) * CD], ident[:]
                    )
                    xT = xq.tile([CD, P], f32, tag="xT", name="xT")
                    act.copy(xT[:], xpt[:])
                    x2T = xq.tile([CD, P], f32, tag="x2T", name="x2T")
                    dve.tensor_mul(x2T[:], xT[:], xpt[:])
                    nc.tensor.matmul(
                        mixp[:],
                        x2T[:],
                        rhs2[:, t * NW : (t + 1) * NW],
                        start=(t == 0),
                        stop=False,
                    )
                    nc.tensor.matmul(
                        mixp[:],
                        xT[:],
                        rhs1[:, t * NW : (t + 1) * NW],
                        start=False,
                        stop=False,
                    )
                nc.tensor.matmul(mixp[:], oner[:], c0[:], start=False, stop=True)

                # ---- epilogue: logsumexp + argmax over k ------------------
                mixs = ep.tile([P, K], f32, tag="mixs", name="mixs")
                act.copy(mixs[:], mixp[:, 0:K])
                mcol = ep.tile([P, 1], f32, tag="mcol", name="mcol")
                dve.tensor_reduce(mcol[:], mixs[:], AX.X, Alu.max)
                msub = ep.tile([P, K], f32, tag="msub", name="msub")
                dve.tensor_scalar_sub(msub[:], mixs[:], mcol[:])
                esb = ep.tile([P, K], f32, tag="esb", name="esb")
                scol = ep.tile([P, 1], f32, tag="scol", name="scol")
                act.activation(esb[:], msub[:], Act.Exp, accum_out=scol[:])
                lns = ep.tile([P, 1], f32, tag="lns", name="lns")
                act.activation(lns[:], scol[:], Act.Ln)
                t2 = ep.tile([P, 1], f32, tag="t2", name="t2")
                dve.scalar_tensor_tensor(
                    t2[:], mcol[:], mixp[:, K : K + 1], lns[:], Alu.add, Alu.add
                )
                lossp = psw.tile([1, 1], f32, tag="tiny", name="tiny")
                nc.tensor.matmul(lossp[:], t2[:], onec[:], start=True, stop=True)
                losss = ep.tile([1, 1], f32, tag="losss", name="losss")
                act.copy(losss[:], lossp[:])
                nc.sync.dma_start(loss_d, losss[:])

                mx8 = ep.tile([P, 8], f32, tag="mx8", name="mx8")
                dve.max(mx8[:], mixs[:])
                ix8 = ep.tile([P, 8], u32, tag="ix8", name="ix8")
                dve.max_index(ix8[:], mx8[:], mixs[:])
                nc.sync.dma_start(idx_d, ix8[:, 0:1])

            if loop_iters is None:
                body()
            else:
                with tc.For_i(0, loop_iters, 1):
                    body()

    return nc


def _get_nc(loop_iters=None):
    key = ("nc", loop_iters)
    if key not in _CACHE:
        nc = _build(loop_iters)
        if not nc.is_finalized():
            nc.finalize()
        _CACHE[key] = nc
    return _CACHE[key]


def _in_maps(x, f_D, f_mu, b_D, b_mu, pi, alpha):
    fstack = np.concatenate(
        [f_D, f_mu, alpha, b_D, b_mu], axis=0
    ).astype(np.float32)  # (98, 2000)
    pi_row = np.ascontiguousarray(pi, dtype=np.float32).reshape(1, K)
    ones_col = np.ones((P, 1), np.float32)
    ones_row = np.ones((1, P), np.float32)
    ident = np.eye(P, dtype=np.float32)
    maps = []
    for c in range(NCORES):
        maps.append(
            {
                "xs": np.ascontiguousarray(x[c * P : (c + 1) * P], dtype=np.float32),
                "fstack": fstack,
                "pi_row": pi_row,
                "ones_col": ones_col,
                "ones_row": ones_row,
                "ident": ident,
            }
        )
    return maps


def _run(maps, loop_iters=None):
    from concourse.bass_utils import run_bass_kernel_spmd

    nc = _get_nc(loop_iters)
    return run_bass_kernel_spmd(nc, maps, list(range(NCORES)))


def kernel(x, f_D, f_mu, b_D, b_mu, pi, alpha):
    x = np.asarray(x)
    maps = _in_maps(
        x,
        np.asarray(f_D),
        np.asarray(f_mu),
        np.asarray(b_D),
        np.asarray(b_mu),
        np.asarray(pi),
        np.asarray(alpha),
    )
    outs = _run(maps).results
    total = np.float64(0.0)
    idx = np.empty((B,), np.int32)
    for c in range(NCORES):
        total += np.float64(outs[c]["loss_acc"][0, 0])
        idx[c * P : (c + 1) * P] = outs[c]["idx_out"][:, 0].astype(np.int32)
    nll = np.float32(-total / B)
    return nll, idx


# revision 12
# speedup vs baseline: 1.0346x; 1.0346x over previous
"""Trainium2 Bass kernel for ConditionalGMM loss (nll mean + argmax assignment).

Math: with univariate Gaussians per dim,
  mix_p[b,k] = log_softmax(pi)[k] + sum_d [ a*f_lp + (1-a)*b_lp ]
Expanding the squared terms turns the big sum over d into two matmuls:
  mix_p'[b,k] = sum_d x^2[b,d]*(-0.5*W2[k,d]) + sum_d x[b,d]*W1[k,d] + c0[k]
with the background (b_*) terms folded in as an extra matmul output column
carrying the k-independent per-sample term Sb[b].
Sharding: data-parallel over batch, 128 rows per core on 8 cores; (K,D)
parameters replicated.  Assignment = argmax_k, nll = -(Sb + logsumexp_k).

Pipelining: the 16 contraction chunks are processed in NG=2 parameter groups
so the ACT/DVE elementwise chains of group 1 overlap group 0's matmuls.
"""

import sys

sys.path.insert(0, "/opt/trn_rl_repo")

import numpy as np

B, D, K = 1024, 2000, 32
NCORES = 8
P = B // NCORES  # 128 batch rows per core
CD, CH = 125, 16  # contraction chunks: D = CH * CD
PR = 96  # stacked param rows: f_D(32) f_mu(32) alpha(32)
NW = K + 1  # matmul rhs cols: 32 mixture cols + 1 background (Sb) col
NG = 2  # parameter pipeline groups
GC = CH // NG  # chunks per group
LOG2PI = float(np.log(2.0 * np.pi))

_CACHE = {}


def _build(loop_iters=None):
    import concourse.bass as bass
    import concourse.bacc as bacc
    import concourse.mybir as mybir
    from concourse.tile import TileContext

    f32 = mybir.dt.float32
    u32 = mybir.dt.uint32
    Act = mybir.ActivationFunctionType
    Alu = mybir.AluOpType
    AX = mybir.AxisListType

    nc = bacc.Bacc()

    xs_d = nc.dram_tensor("xs", [P, D], f32, kind="ExternalInput").ap()
    fst_d = nc.dram_tensor("fstack", [PR, D], f32, kind="ExternalInput").ap()
    pi_d = nc.dram_tensor("pi_row", [1, K], f32, kind="ExternalInput").ap()
    onec_d = nc.dram_tensor("ones_col", [P, 1], f32, kind="ExternalInput").ap()
    oner_d = nc.dram_tensor("ones_row", [1, P], f32, kind="ExternalInput").ap()
    id_d = nc.dram_tensor("ident", [P, P], f32, kind="ExternalInput").ap()
    bt_d = nc.dram_tensor("bT", [CD, 2 * CH], f32, kind="ExternalInput").ap()
    loss_d = nc.dram_tensor("loss_acc", [1, 1], f32, kind="ExternalOutput").ap()
    idx_d = nc.dram_tensor("idx_out", [P, 1], u32, kind="ExternalOutput").ap()

    act = nc.scalar
    dve = nc.vector

    with TileContext(nc) as tc:
        with (
            tc.tile_pool(name="const", bufs=1) as cp,
            tc.tile_pool(name="par", bufs=1) as pp,
            tc.tile_pool(name="xp", bufs=1) as xp,
            tc.tile_pool(name="xs2", bufs=2) as xq,
            tc.tile_pool(name="ep", bufs=1) as ep,
            tc.tile_pool(name="pst", bufs=3, space="PSUM") as pst,
            tc.tile_pool(name="psm", bufs=1, space="PSUM") as psm,
            tc.tile_pool(name="psw", bufs=1, space="PSUM") as psw,
        ):
            # ---- one-time constants (ACT HWDGE queue) ---------------------
            ident = cp.tile([P, P], f32, tag="ident", name="ident")
            act.dma_start(ident[:], id_d)
            onec = cp.tile([P, 1], f32, tag="onec", name="onec")
            act.dma_start(onec[:], onec_d)
            oner = cp.tile([1, P], f32, tag="oner", name="oner")
            act.dma_start(oner[:], oner_d)

            # warm the ACT natural_log_exp table ASAP (no DMA dependency)
            wsrc = cp.tile([1, 1], f32, tag="wsrc", name="wsrc")
            nc.gpsimd.memset(wsrc[:], 0)
            warm = cp.tile([1, 1], f32, tag="warm", name="warm")
            act.activation(warm[:], wsrc[:], Act.Exp)

            def body():
                # ---- input DMAs: params on ACT queue, x on SP queue -------
                pi_r = cp.tile([1, K], f32, tag="pi", name="pi")
                nc.sync.dma_start(pi_r[:], pi_d)
                bT = pp.tile([CD, 2 * CH], f32, tag="bT", name="bT")
                nc.gpsimd.dma_start(bT[:], bt_d)
                praw = []
                for j in range(NG):
                    t = pp.tile(
                        [PR, D // NG], f32, tag=f"praw{j}", name=f"praw{j}"
                    )
                    nc.sync.dma_start(
                        t[:], fst_d[:, j * (D // NG) : (j + 1) * (D // NG)]
                    )
                    praw.append(t)
                xnat = []
                for j in range(4):
                    t = xp.tile([P, 500], f32, tag=f"xnat{j}", name=f"xnat{j}")
                    nc.gpsimd.dma_start(t[:], xs_d[:, j * 500 : (j + 1) * 500])
                    xnat.append(t)

                # ---- background (b_*) vectors once, [CD, CH] --------------
                bDTa = bT[:, 0:CH]
                bmuTa = bT[:, CH : 2 * CH]
                def smlb(tag):
                    return pp.tile([CD, CH], f32, tag=tag, name=tag)
                bve = smlb("bve")
                act.activation(bve[:], bDTa, Act.Exp)
                bv = smlb("bv")
                act.activation(bv[:], bve[:], Act.Ln, bias=1.0)
                rb_a = smlb("rb")
                dve.reciprocal(rb_a[:], bv[:])
                lvb = smlb("lvb")
                act.activation(lvb[:], bv[:], Act.Ln)
                hrb_a = smlb("hrb")  # 0.5 * r_b
                nc.gpsimd.tensor_scalar_mul(hrb_a[:], rb_a[:], 0.5)
                nbr_a = smlb("nbr")  # -b_mu * r_b
                dve.scalar_tensor_tensor(
                    nbr_a[:], bmuTa, -1.0, rb_a[:], Alu.mult, Alu.mult
                )
                b2 = smlb("b2")
                nc.gpsimd.tensor_mul(b2[:], bmuTa, bmuTa)
                q2 = smlb("q2")  # b_mu^2 * r_b
                dve.tensor_mul(q2[:], b2[:], rb_a[:])
                nqb_a = smlb("nqb")  # -(b_mu^2 r_b + ln b_var) = -qb'
                nqrow = pp.tile([CD, 1], f32, tag="nqrow", name="nqrow")
                dve.scalar_tensor_tensor(
                    nqb_a[:],
                    q2[:],
                    -1.0,
                    lvb[:],
                    Alu.mult,
                    Alu.subtract,
                    accum_out=nqrow[:],
                )

                # ---- per-group param transpose + elementwise pipeline -----
                # parTg[:, u*PR + j] = param row j at dim d = (g*GC+u)*CD + p
                parT = []
                rhs2 = []
                rhs1 = []
                A0f = []
                for g in range(NG):
                    parTg = pp.tile(
                        [CD, GC * PR], f32, tag=f"parT{g}", name=f"parT{g}"
                    )
                    parT.append(parTg)
                    for h in range(2):  # 4 transposed chunks per PSUM bank
                        pps = pst.tile([CD, 4 * PR], f32, tag="tp", name="tp")
                        for q in range(4):
                            u = 4 * h + q  # chunk within group
                            nc.tensor.transpose(
                                pps[:, q * PR : (q + 1) * PR],
                                praw[g][:, u * CD : (u + 1) * CD],
                                ident[0:PR, 0:PR],
                            )
                        act.copy(
                            parTg[:, h * 4 * PR : (h + 1) * 4 * PR], pps[:]
                        )

                    def pview(j0, w):
                        a = parTg[:]
                        return bass.AP(
                            a.tensor, a.offset + j0, [a.ap[0], [PR, GC], [1, w]]
                        )

                    fDT = pview(0, K)
                    fmuT = pview(K, K)
                    alT = pview(2 * K, K)

                    def big(tag):
                        return pp.tile(
                            [CD, GC * K], f32, tag=f"{tag}{g}", name=f"{tag}{g}"
                        )

                    def v3(ap):
                        return bass.AP(
                            ap.tensor, ap.offset, [ap.ap[0], [K, GC], [1, K]]
                        )

                    def bc0(ap):
                        return bass.AP(
                            ap.tensor, ap.offset, [ap.ap[0], [1, GC], [0, K]]
                        )

                    # f_var = softplus(f_D) = ln(exp(f_D)+1); a = sigmoid(10a)
                    eA = big("eA")
                    act.activation(v3(eA[:]), fDT, Act.Exp)
                    fv = big("fv")
                    act.activation(fv[:], eA[:], Act.Ln, bias=1.0)
                    rf = big("rf")
                    dve.reciprocal(rf[:], fv[:])
                    lvf = big("lvf")
                    act.activation(lvf[:], fv[:], Act.Ln)
                    eB = big("eB")
                    act.activation(v3(eB[:]), alT, Act.Exp, scale=-10.0)
                    s2 = big("s2")
                    nc.gpsimd.tensor_scalar_add(s2[:], eB[:], 1.0)
                    aS = big("aS")
                    dve.reciprocal(aS[:], s2[:])
                    fm2 = big("fm2")
                    nc.gpsimd.tensor_mul(v3(fm2[:]), fmuT, fmuT)
                    P2 = big("P2")
                    dve.tensor_mul(P2[:], aS[:], rf[:])
                    P1 = big("P1")
                    dve.tensor_mul(P1[:], P2[:], fmuT)

                    rb = rb_a[:, g * GC : (g + 1) * GC]
                    hrb = hrb_a[:, g * GC : (g + 1) * GC]
                    nbr = nbr_a[:, g * GC : (g + 1) * GC]
                    nqb = nqb_a[:, g * GC : (g + 1) * GC]

                    # A0f = a*(fmu^2*rf + ln fv - qb')  (W0 integrand)
                    fm2rf = big("fm2rf")
                    nc.gpsimd.tensor_mul(fm2rf[:], fm2[:], rf[:])
                    inner = big("inner")
                    nc.gpsimd.tensor_add(inner[:], fm2rf[:], lvf[:])
                    inner2 = big("inner2")
                    dve.tensor_add(v3(inner2[:]), v3(inner[:]), bc0(nqb))
                    A0g = big("A0f")
                    nc.gpsimd.tensor_mul(A0g[:], aS[:], inner2[:])
                    A0f.append(A0g)

                    # m1 = a*(0.5 r_b), m2 = a*(-b_mu r_b)  (bcast over k)
                    m1 = big("m1")
                    dve.tensor_mul(v3(m1[:]), v3(aS[:]), bc0(hrb))
                    m2 = big("m2")
                    dve.tensor_mul(v3(m2[:]), v3(aS[:]), bc0(nbr))

                    # matmul rhs tensors [CD, GC*NW]
                    r2 = pp.tile([CD, GC * NW], f32, tag=f"rhs2{g}", name=f"rhs2{g}")
                    r1 = pp.tile([CD, GC * NW], f32, tag=f"rhs1{g}", name=f"rhs1{g}")
                    rhs2.append(r2)
                    rhs1.append(r1)

                    def wview(ap):
                        return bass.AP(
                            ap.tensor, ap.offset, [ap.ap[0], [NW, GC], [1, K]]
                        )

                    def bcolv(ap):
                        return bass.AP(
                            ap.tensor, ap.offset + K, [ap.ap[0], [NW, GC]]
                        )

                    # rhs2 W-cols = -0.5*P2 + m1 ; b-col = -0.5*r_b
                    dve.scalar_tensor_tensor(
                        wview(r2[:]), v3(P2[:]), -0.5, v3(m1[:]), Alu.mult, Alu.add
                    )
                    nc.gpsimd.tensor_scalar_mul(bcolv(r2[:]), rb, -0.5)
                    # rhs1 W-cols = P1 + m2 ; b-col = +b_mu*r_b = -nbr
                    dve.scalar_tensor_tensor(
                        wview(r1[:]), v3(P1[:]), 1.0, v3(m2[:]), Alu.mult, Alu.add
                    )
                    nc.gpsimd.tensor_scalar_mul(bcolv(r1[:]), nbr, -1.0)

                # ---- W0[k] = sum_d A0f  (PSUM [K,1] via ones matmuls) -----
                w0p = psw.tile([K, 1], f32, tag="w0p", name="w0p")
                for g in range(NG):
                    for u in range(GC):
                        nc.tensor.matmul(
                            w0p[:],
                            A0f[g][:, u * K : (u + 1) * K],
                            onec[0:CD, :],
                            start=(g == 0 and u == 0),
                            stop=(g == NG - 1 and u == GC - 1),
                        )
                w0s = ep.tile([K, 1], f32, tag="w0s", name="w0s")
                act.copy(w0s[:], w0p[:])
                w0r = psw.tile([1, K], f32, tag="w0r", name="w0r")
                nc.tensor.transpose(w0r[:], w0s[:], ident[0:K, 0:K])

                # sum_d nqb (PSUM [1,1]) for the background constant
                s0p = psw.tile([1, 1], f32, tag="tiny", name="tiny")
                nc.tensor.matmul(
                    s0p[:], nqrow[:], onec[0:CD, :], start=True, stop=True
                )

                # ---- c0 row [1, NW]: log_softmax(pi) - 0.5*W0 | s0 --------
                mx = ep.tile([1, 1], f32, tag="mx", name="mx")
                dve.tensor_reduce(mx[:], pi_r[:], AX.X, Alu.max)
                g1 = ep.tile([1, K], f32, tag="g1", name="g1")
                dve.tensor_scalar_sub(g1[:], pi_r[:], mx[:])
                epi = ep.tile([1, K], f32, tag="epi", name="epi")
                spi = ep.tile([1, 1], f32, tag="spi", name="spi")
                act.activation(epi[:], g1[:], Act.Exp, accum_out=spi[:])
                lnspi = ep.tile([1, 1], f32, tag="lnspi", name="lnspi")
                act.activation(lnspi[:], spi[:], Act.Ln)
                g2 = ep.tile([1, K], f32, tag="g2", name="g2")
                dve.tensor_scalar_sub(g2[:], g1[:], lnspi[:])
                c0 = ep.tile([1, NW], f32, tag="c0", name="c0")
                dve.scalar_tensor_tensor(
                    c0[:, 0:K], w0r[:], -0.5, g2[:], Alu.mult, Alu.add
                )
                act.activation(
                    c0[:, K : K + 1],
                    s0p[:],
                    Act.Copy,
                    bias=-0.5 * D * LOG2PI,
                    scale=0.5,
                )

                # ---- x: transpose 4 chunks/bank, square, matmul -----------
                mixp = psm.tile([P, NW], f32, tag="mix", name="mix")
                for xg in range(4):
                    xps = pst.tile([CD, 512], f32, tag="tp", name="tp")
                    for q in range(4):
                        nc.tensor.transpose(
                            xps[:, q * P : (q + 1) * P],
                            xnat[xg][:, q * CD : (q + 1) * CD],
                            ident[:],
                        )
                    xT = xq.tile([CD, 512], f32, tag="xT", name="xT")
                    act.copy(xT[:], xps[:])
                    x2T = xq.tile([CD, 512], f32, tag="x2T", name="x2T")
                    dve.tensor_mul(x2T[:], xT[:], xps[:])
                    for q in range(4):
                        t = 4 * xg + q
                        g, u = t // GC, t % GC
                        nc.tensor.matmul(
                            mixp[:],
                            x2T[:, q * P : (q + 1) * P],
                            rhs2[g][:, u * NW : (u + 1) * NW],
                            start=(t == 0),
                            stop=False,
                        )
                        nc.tensor.matmul(
                            mixp[:],
                            xT[:, q * P : (q + 1) * P],
                            rhs1[g][:, u * NW : (u + 1) * NW],
                            start=False,
                            stop=False,
                        )
                nc.tensor.matmul(mixp[:], oner[:], c0[:], start=False, stop=True)

                # ---- epilogue: logsumexp (PSUM-direct) + argmax -----------
                mcol = ep.tile([P, 1], f32, tag="mcol", name="mcol")
                dve.tensor_reduce(mcol[:], mixp[:, 0:K], AX.X, Alu.max)
                msub = ep.tile([P, K], f32, tag="msub", name="msub")
                dve.tensor_scalar_sub(msub[:], mixp[:, 0:K], mcol[:])
                esb = ep.tile([P, K], f32, tag="esb", name="esb")
                scol = ep.tile([P, 1], f32, tag="scol", name="scol")
                act.activation(esb[:], msub[:], Act.Exp, accum_out=scol[:])
                lns = ep.tile([P, 1], f32, tag="lns", name="lns")
                act.activation(lns[:], scol[:], Act.Ln)
                t2 = ep.tile([P, 1], f32, tag="t2", name="t2")
                dve.scalar_tensor_tensor(
                    t2[:], mcol[:], mixp[:, K : K + 1], lns[:], Alu.add, Alu.add
                )
                lossp = psw.tile([1, 1], f32, tag="tiny", name="tiny")
                nc.tensor.matmul(lossp[:], t2[:], onec[:], start=True, stop=True)
                losss = ep.tile([1, 1], f32, tag="losss", name="losss")
                act.copy(losss[:], lossp[:])
                act.dma_start(loss_d, losss[:])

                # argmax branch (max/max_index need SBUF inputs)
                mixs = ep.tile([P, K], f32, tag="mixs", name="mixs")
                act.copy(mixs[:], mixp[:, 0:K])
                mx8 = ep.tile([P, 8], f32, tag="mx8", name="mx8")
                dve.max(mx8[:], mixs[:])
                ix8 = ep.tile([P, 8], u32, tag="ix8", name="ix8")
                dve.max_index(ix8[:], mx8[:], mixs[:])
                nc.sync.dma_start(idx_d, ix8[:, 0:1])

            if loop_iters is None:
                body()
            else:
                with tc.For_i(0, loop_iters, 1):
                    body()

    return nc


def _get_nc(loop_iters=None):
    key = ("nc", loop_iters)
    if key not in _CACHE:
        nc = _build(loop_iters)
        if not nc.is_finalized():
            nc.finalize()
        _CACHE[key] = nc
    return _CACHE[key]


def _in_maps(x, f_D, f_mu, b_D, b_mu, pi, alpha):
    fstack = np.concatenate([f_D, f_mu, alpha], axis=0).astype(
        np.float32
    )  # (96, 2000)
    bT = np.concatenate(
        [
            np.asarray(b_D, np.float32).reshape(CH, CD).T,
            np.asarray(b_mu, np.float32).reshape(CH, CD).T,
        ],
        axis=1,
    )  # (125, 32): transposed background params (pure relayout)
    pi_row = np.ascontiguousarray(pi, dtype=np.float32).reshape(1, K)
    ones_col = np.ones((P, 1), np.float32)
    ones_row = np.ones((1, P), np.float32)
    ident = np.eye(P, dtype=np.float32)
    maps = []
    for c in range(NCORES):
        maps.append(
            {
                "xs": np.ascontiguousarray(x[c * P : (c + 1) * P], dtype=np.float32),
                "fstack": fstack,
                "pi_row": pi_row,
                "bT": bT,
                "ones_col": ones_col,
                "ones_row": ones_row,
                "ident": ident,
            }
        )
    return maps


def _run(maps, loop_iters=None):
    from concourse.bass_utils import run_bass_kernel_spmd

    nc = _get_nc(loop_iters)
    return run_bass_kernel_spmd(nc, maps, list(range(NCORES)))


def kernel(x, f_D, f_mu, b_D, b_mu, pi, alpha):
    x = np.asarray(x)
    maps = _in_maps(
        x,
        np.asarray(f_D),
        np.asarray(f_mu),
        np.asarray(b_D),
        np.asarray(b_mu),
        np.asarray(pi),
        np.asarray(alpha),
    )
    outs = _run(maps).results
    total = np.float64(0.0)
    idx = np.empty((B,), np.int32)
    for c in range(NCORES):
        total += np.float64(outs[c]["loss_acc"][0, 0])
        idx[c * P : (c + 1) * P] = outs[c]["idx_out"][:, 0].astype(np.int32)
    nll = np.float32(-total / B)
    return nll, idx
